# revision 28
# baseline (speedup 1.0000x reference)
"""Trainium2 Bass kernel for windowed multi-head attention (Swin-style block).

Reference computation (per batch window b of 128, N=196 tokens, C=768, H=12 heads):
    qkv  = x @ qkv_w.T + [q_bias, 0, v_bias]
    q,k,v = split(qkv);  attn = softmax(q*scale @ k.T + rel_pos_bias)
    out  = (attn @ v) @ proj_w.T + proj_b

Sharding: data-parallel over batch across 8 cores (16 windows/core).

Per-core kernel layout strategy (all matmuls consume operands in natural layout,
softmax runs in "transposed" space so no on-the-fly attention transposes):
  - x [196,768] is PE-transposed once to xT [768,196] (lhsT/rhs source).
  - Q^T,K^T [c',t] = W^T chunks (lhsT) x xT (rhs); V [t,c'] = xT (lhsT) x W^T (rhs).
  - S^T[j,i] = K^T-head (lhsT) x Q^T-head (rhs), K=64 contraction; heads pairs
    are row-tiled (partition base 0/64) so two K=64 matmuls share the PE array.
  - E^T = exp(0.125*S^T) * exp(bias)^T  (ACT exp from PSUM, DVE multiply with a
    host-precomputed exp(bias) table -- exp(a+b)=exp(a)exp(b)).
  - O^T[d,i] per head = [V-head | ones] (lhsT) x E^T (rhs): row 64 of the psum
    output is the softmax denominator for free (ones column in V).
  - per head pair: DVE copies the two denominator rows out, one reciprocal,
    one GPSIMD partition_broadcast to [64,2,196]; normalization is fused into
    the O^T PSUM->SBUF eviction (DVE multiply by the broadcast reciprocal).
  - y = O^T chunks (lhsT) x P^T (rhs) + proj_b (DVE add with broadcast bias).
Hardware notes: matmuls at different partition bases must not share a psum
bank (device-fatal); K=64 head-pair matmuls alternate PE row groups 0/64.
"""

import sys

import numpy as np

if "/opt/trn_rl_repo" not in sys.path:
    sys.path.insert(0, "/opt/trn_rl_repo")

import concourse.bass as bass  # noqa: E402
import concourse.mybir as mybir  # noqa: E402
import concourse.tile as tile  # noqa: E402
from concourse import bacc  # noqa: E402
from concourse import bass_utils  # noqa: E402
from concourse.masks import make_identity  # noqa: E402

# Problem shapes (hardcoded; kernel.py must be self-contained).
B, N, C = 128, 196, 768
H, HD = 12, 64
WS = 14
NCORES = 8
BW = B // NCORES  # 16 windows per core
NPAIRS = BW // 2
JC = 98  # j/t chunk size (2 chunks per 196-token window)
F32 = mybir.dt.float32
BF16 = mybir.dt.bfloat16
NP_BF16 = mybir.dt.np(BF16)
SCALE = HD ** -0.5  # 0.125


def _relative_position_index(ws: int) -> np.ndarray:
    coords = np.stack(np.meshgrid(np.arange(ws), np.arange(ws), indexing="ij"))
    flat = coords.reshape(2, -1)
    rel = flat[:, :, None] - flat[:, None, :]
    rel = rel.transpose(1, 2, 0).copy()
    rel[..., 0] += ws - 1
    rel[..., 1] += ws - 1
    rel[..., 0] *= 2 * ws - 1
    return rel.sum(-1)  # [N, N] int


def _build_kernel_body(ctx, tc, aps, reps=1):
    nc = tc.nc
    x_d = aps["x_sh"]
    wT_d = aps["wT"]
    pT_d = aps["pT"]
    qb_d = aps["qb"]
    vb_d = aps["vb"]
    pb_d = aps["pb"]
    eb_d = aps["expBT"]
    y_d = aps["y_sh"]

    const = ctx.enter_context(tc.tile_pool(name="const", bufs=1))

    # ---- resident constants ----
    w_sb = const.tile([128, 6, 3 * C], BF16)  # W^T: [c%128, c//128, c']
    nc.sync.dma_start(out=w_sb, in_=wT_d.rearrange("(a p) m -> p a m", p=128))
    pT_sb = const.tile([128, 6, C], BF16)
    nc.sync.dma_start(out=pT_sb, in_=pT_d.rearrange("(a p) m -> p a m", p=128))
    qb_sb = const.tile([128, 6], F32)
    nc.sync.dma_start(out=qb_sb, in_=qb_d.rearrange("(a p) -> p a", p=128))
    def _bcast(src, parts):
        return bass.AP(tensor=src.tensor, offset=src.offset,
                       ap=[[0, parts]] + list(src.ap))

    vb_bc = const.tile([128, C], F32)  # v_bias broadcast along partitions
    nc.sync.dma_start(out=vb_bc, in_=_bcast(vb_d, 128))
    pb_bc = const.tile([128, C], F32)
    nc.sync.dma_start(out=pb_bc, in_=_bcast(pb_d, 128))
    # 8*bias^T table: [j%98, h, (j//98)*196+i]; injected into the S psum
    # via an identity-copy matmul so exp(0.125*(S+8b)) = exp(S/8 + b)
    b8_sb = const.tile([JC, H, 2 * N], BF16)
    nc.sync.dma_start(out=b8_sb, in_=eb_d.rearrange("p (h m) -> p h m", h=H))
    ident = const.tile([128, 128], BF16)
    make_identity(nc, ident)

    # ---- pools ----
    xin = ctx.enter_context(tc.tile_pool(name="xin", bufs=4))
    xt = ctx.enter_context(tc.tile_pool(name="xt", bufs=2))
    qk = ctx.enter_context(tc.tile_pool(name="qk", bufs=2))
    vpool = ctx.enter_context(tc.tile_pool(name="vpool", bufs=4))
    epool = ctx.enter_context(tc.tile_pool(name="epool", bufs=4))
    opool = ctx.enter_context(tc.tile_pool(name="opool", bufs=4))
    rpool = ctx.enter_context(tc.tile_pool(name="rpool", bufs=4))
    rbc = ctx.enter_context(tc.tile_pool(name="rbc", bufs=4))
    ypool = ctx.enter_context(tc.tile_pool(name="ypool", bufs=2))
    ps_mm = ctx.enter_context(tc.tile_pool(name="ps_mm", bufs=2, space="PSUM"))
    ps_s = ctx.enter_context(tc.tile_pool(name="ps_s", bufs=3, space="PSUM"))
    ps_od = ctx.enter_context(tc.tile_pool(name="ps_od", bufs=3, space="PSUM"))

    # Software pipeline over the global pair stream: while pair P's
    # attention chains run (PE-starved dependency chains), the next pair's
    # transpose/QKV/V matmul groups are woven between them in program order
    # so the in-order PE queue always has independent work.
    state = {}

    def emit_load(P):
        pi = P % NPAIRS
        st = {"wins": (2 * pi, 2 * pi + 1), "xa": [], "xb": []}
        for w in st["wins"]:
            ta = xin.tile([128, C], BF16, tag="xa", name="ta")
            nc.sync.dma_start(out=ta, in_=x_d[w, 0:128, :])
            st["xa"].append(ta)
            tb = xin.tile([128, C], BF16, tag="xb", name="tb")  # rows 0:68
            nc.sync.dma_start(out=tb[0:68, :], in_=x_d[w, 128:196, :])
            st["xb"].append(tb)
        st["xT"] = xt.tile([128, 6, 2 * N], BF16, name="xT")
        st["qk"] = qk.tile([128, 12, 2 * N], BF16, name="qk_sb")
        st["v"] = [
            vpool.tile([128, 2, H, HD + 1], BF16, tag="v", name="vt")
            for _ in range(2)
        ]
        st["oc"] = [
            opool.tile([128, 6, N], BF16, tag="oc", name="oc") for _ in range(2)
        ]
        state[P] = st

    def emit_vms(P, wi):
        nc.gpsimd.memset(state[P]["v"][wi][0:JC, :, :, HD : HD + 1], 1.0)

    def emit_transpose(P, ci):
        st = state[P]
        pt = ps_mm.tile([128, 512], BF16, tag="mm", name="pt")
        for wi in range(2):
            nc.tensor.transpose(
                pt[:, wi * N : wi * N + 128],
                st["xa"][wi][:, ci * 128 : (ci + 1) * 128],
                ident,
            )
            nc.tensor.transpose(
                pt[:, wi * N + 128 : wi * N + N],
                st["xb"][wi][0:68, ci * 128 : (ci + 1) * 128],
                ident[0:68, 0:68],
            )
        nc.scalar.copy(out=st["xT"][:, ci, :], in_=pt[:, 0 : 2 * N])

    def emit_qk(P, cp):
        st = state[P]
        ps = ps_mm.tile([128, 512], F32, tag="mm", name="ps")
        for ck in range(6):
            nc.tensor.matmul(
                ps[:, 0 : 2 * N],
                w_sb[:, ck, cp * 128 : (cp + 1) * 128],
                st["xT"][:, ck, :],
                start=(ck == 0),
                stop=(ck == 5),
            )
        if cp < 6:  # Q: add q_bias (per-partition bias) on ACT
            nc.scalar.activation(
                out=st["qk"][:, cp, :], in_=ps[:, 0 : 2 * N],
                func=mybir.ActivationFunctionType.Identity,
                bias=qb_sb[:, cp : cp + 1],
            )
        else:  # K: plain copy on ACT
            nc.scalar.copy(out=st["qk"][:, cp, :], in_=ps[:, 0 : 2 * N])

    def emit_v(P, wi, tck, c0, nn):
        st = state[P]
        ps = ps_mm.tile([128, 512], F32, tag="mm", name="ps")
        for ck in range(6):
            nc.tensor.matmul(
                ps[0:JC, 0:nn],
                st["xT"][:, ck, wi * N + tck * JC : wi * N + (tck + 1) * JC],
                w_sb[:, ck, 2 * C + c0 : 2 * C + c0 + nn],
                start=(ck == 0),
                stop=(ck == 5),
            )
        h0 = c0 // HD
        nh = nn // HD
        nc.vector.tensor_add(
            out=st["v"][wi][0:JC, tck, h0 : h0 + nh, 0:HD],
            in0=ps[0:JC, 0:nn].rearrange("p (h d) -> p h d", d=HD),
            in1=vb_bc[0:JC, c0 : c0 + nn].rearrange("p (h d) -> p h d", d=HD),
        )

    def emit_att(P, g, wi):
        st = state[P]
        woff = wi * N
        oc = st["oc"][wi]
        qk_sb = st["qk"]
        e2 = epool.tile([JC, 2, 2, N], BF16, tag="e", name="e2")  # [j,hh,jc,i]
        pso = ps_od.tile([HD + 1, 2, N], F32, tag="od", name="pso")
        # S phase for both heads first, so exp(hh=0) overlaps S(hh=1) and
        # the O matmuls never head-of-line-block the PE queue
        for hh in range(2):
            h = 2 * g + hh
            prow = (h % 2) * 64
            pss = ps_s.tile([128, 512], F32, tag="s", name="pss")
            # seed psum with 8*bias^T (identity-copy matmul, both jc
            # column halves at once), then accumulate S^T on top; exp's
            # 0.125 scale folds both
            nc.tensor.matmul(
                pss[0:JC, 0 : 2 * N],
                ident[0:JC, 0:JC],
                b8_sb[:, h, :],
                start=True,
                stop=False,
            )
            for jc in range(2):
                nc.tensor.matmul(
                    pss[0:JC, jc * N : (jc + 1) * N],
                    qk_sb[prow : prow + 64, 6 + h // 2,
                          woff + jc * JC : woff + (jc + 1) * JC],
                    qk_sb[prow : prow + 64, h // 2, woff : woff + N],
                    start=False,
                    stop=(jc == 1),
                )
            nc.scalar.activation(
                out=e2[:, hh, :, :],
                in_=pss[0:JC, 0 : 2 * N].rearrange("p (a n) -> p a n", a=2),
                func=mybir.ActivationFunctionType.Exp,
                scale=SCALE,
            )
        # O^T (+denominator row 64 via the V ones column); both heads
        # share one psum tile (same partition base, disjoint free)
        for hh in range(2):
            h = 2 * g + hh
            for jc in range(2):
                nc.tensor.matmul(
                    pso[:, hh, :],
                    st["v"][wi][0:JC, jc, h, :],
                    e2[0:JC, hh, jc, :],
                    start=(jc == 0),
                    stop=(jc == 1),
                )
        r2 = rpool.tile([1, 2, N], F32, tag="r", name="r2")
        nc.vector.reciprocal(out=r2, in_=pso[HD : HD + 1, :, :])
        rb = rbc.tile([64, 2, N], F32, tag="rb", name="rb")
        nc.gpsimd.partition_broadcast(rb, r2)
        nc.vector.tensor_mul(oc[0:64, g, :], pso[0:HD, 0, :], rb[:, 0, :])
        nc.vector.tensor_mul(oc[64:128, g, :], pso[0:HD, 1, :], rb[:, 1, :])

    def emit_proj(P, wi, tck):
        st = state[P]
        w = st["wins"][wi]
        oc = st["oc"][wi]
        y_t = ypool.tile([128, C], BF16, tag="y", name="y_t")
        for c0, nn in ((0, 512), (512, 256)):
            ps = ps_mm.tile([128, 512], F32, tag="mm", name="ps")
            for ck in range(6):
                nc.tensor.matmul(
                    ps[0:JC, 0:nn],
                    oc[:, ck, tck * JC : (tck + 1) * JC],
                    pT_sb[:, ck, c0 : c0 + nn],
                    start=(ck == 0),
                    stop=(ck == 5),
                )
            nc.vector.tensor_add(
                out=y_t[0:JC, c0 : c0 + nn],
                in0=ps[0:JC, 0:nn],
                in1=pb_bc[0:JC, c0 : c0 + nn],
            )
        nc.sync.dma_start(
            out=y_d[w, tck * JC : (tck + 1) * JC, :], in_=y_t[0:JC, :]
        )

    def fillers(P):
        fs = []
        for wi in range(2):
            fs.append(lambda wi=wi: emit_vms(P, wi))
        for ci in range(6):
            fs.append(lambda ci=ci: emit_transpose(P, ci))
        for cp in range(12):
            fs.append(lambda cp=cp: emit_qk(P, cp))
        for wi in range(2):
            for tck in range(2):
                for c0, nn in ((0, 512), (512, 256)):
                    fs.append(
                        lambda wi=wi, tck=tck, c0=c0, nn=nn:
                        emit_v(P, wi, tck, c0, nn))
        return fs

    total = reps * NPAIRS
    emit_load(0)
    for f in fillers(0):
        f()
    for P in range(total):
        nf = []
        if P + 1 < total:
            emit_load(P + 1)
            nf = fillers(P + 1)
        chains = [(g, wi) for g in range(6) for wi in range(2)]
        fi = 0
        for idx, (g, wi) in enumerate(chains):
            emit_att(P, g, wi)
            want = (idx + 1) * len(nf) // len(chains)
            while fi < want:
                nf[fi]()
                fi += 1
        while fi < len(nf):
            nf[fi]()
            fi += 1
        for wi in range(2):
            for tck in range(2):
                emit_proj(P, wi, tck)
        del state[P]


def build_program(reps=1):
    """Build + compile the per-core Bass program. Returns the Bacc instance."""
    nc = bacc.Bacc(
        "TRN2",
        target_bir_lowering=False,
        debug=False,
        enable_asserts=False,
        num_devices=NCORES,
    )
    aps = {
        "x_sh": nc.dram_tensor("x_sh", [BW, N, C], BF16, kind="ExternalInput").ap(),
        "wT": nc.dram_tensor("wT", [C, 3 * C], BF16, kind="ExternalInput").ap(),
        "pT": nc.dram_tensor("pT", [C, C], BF16, kind="ExternalInput").ap(),
        "qb": nc.dram_tensor("qb", [C], F32, kind="ExternalInput").ap(),
        "vb": nc.dram_tensor("vb", [C], F32, kind="ExternalInput").ap(),
        "pb": nc.dram_tensor("pb", [C], F32, kind="ExternalInput").ap(),
        "expBT": nc.dram_tensor(
            "expBT", [JC, H * 2 * N], BF16, kind="ExternalInput").ap(),
        "y_sh": nc.dram_tensor("y_sh", [BW, N, C], BF16, kind="ExternalOutput").ap(),
    }

    from contextlib import ExitStack

    with tile.TileContext(nc) as tc:
        with ExitStack() as ctx:
            _build_kernel_body(ctx, tc, aps, reps=reps)
    nc.compile()
    return nc


_CACHED = {}


def _get_program(reps=1):
    key = f"nc{reps}"
    if key not in _CACHED:
        _CACHED[key] = build_program(reps=reps)
    return _CACHED[key]


def host_prep(qkv_w, q_bias, v_bias, rpb_table, proj_w, proj_b):
    """Host-side constant layout prep (shared across cores)."""
    idx = _relative_position_index(WS)  # [N, N] ints
    bias = rpb_table[idx.reshape(-1)].reshape(N, N, H)  # [i, j, h]
    b8 = 8.0 * bias.astype(np.float32)
    # expBT[r, h, jc*N + i] = 8*bias[i, jc*JC + r, h]
    e = b8.transpose(2, 1, 0).reshape(H, 2, JC, N)  # [h, jc, r, i]
    expBT = np.ascontiguousarray(e.transpose(2, 0, 1, 3)).reshape(JC, H * 2 * N)
    return {
        "wT": np.ascontiguousarray(qkv_w.T).astype(NP_BF16),
        "pT": np.ascontiguousarray(proj_w.T).astype(NP_BF16),
        "qb": np.ascontiguousarray(q_bias, np.float32),
        "vb": np.ascontiguousarray(v_bias, np.float32),
        "pb": np.ascontiguousarray(proj_b, np.float32),
        "expBT": expBT.astype(NP_BF16),
    }


def make_in_maps(x, qkv_w, q_bias, v_bias, rpb_table, proj_w, proj_b):
    shared = host_prep(qkv_w, q_bias, v_bias, rpb_table, proj_w, proj_b)
    in_maps = []
    x_bf = np.asarray(x, np.float32).astype(NP_BF16)
    for ci in range(NCORES):
        m = dict(shared)
        m["x_sh"] = np.ascontiguousarray(x_bf[ci * BW : (ci + 1) * BW])
        in_maps.append(m)
    return in_maps


def kernel(x, qkv_w, q_bias, v_bias, rpb_table, proj_w, proj_b, _trace=False):
    """Full-input entry point: shards over 8 NeuronCores, returns full output."""
    nc = _get_program()
    in_maps = make_in_maps(x, qkv_w, q_bias, v_bias, rpb_table, proj_w, proj_b)
    res = bass_utils.run_bass_kernel_spmd(
        nc, in_maps, core_ids=list(range(NCORES)), trace=_trace)
    out = np.concatenate(
        [res.results[i]["y_sh"] for i in range(NCORES)], axis=0
    ).astype(np.float32)
    if _trace:
        return out, res
    return out



# revision 31
# speedup vs baseline: 2.4563x; 2.4563x over previous
"""Trainium2 Bass kernel for windowed multi-head attention (Swin-style block).

Reference computation (per batch window b of 128, N=196 tokens, C=768, H=12 heads):
    qkv  = x @ qkv_w.T + [q_bias, 0, v_bias]
    q,k,v = split(qkv);  attn = softmax(q*scale @ k.T + rel_pos_bias)
    out  = (attn @ v) @ proj_w.T + proj_b

Sharding: data-parallel over batch across 8 cores (16 windows/core).

Per-core kernel layout strategy (all matmuls consume operands in natural layout,
softmax runs in "transposed" space so no on-the-fly attention transposes):
  - x [196,768] is PE-transposed once to xT [768,196] (lhsT/rhs source).
  - Q^T,K^T [c',t] = W^T chunks (lhsT) x xT (rhs); V [t,c'] = xT (lhsT) x W^T (rhs).
  - S^T[j,i] = K^T-head (lhsT) x Q^T-head (rhs), K=64 contraction; heads pairs
    are row-tiled (partition base 0/64) so two K=64 matmuls share the PE array.
  - E^T = exp(0.125*S^T) * exp(bias)^T  (ACT exp from PSUM, DVE multiply with a
    host-precomputed exp(bias) table -- exp(a+b)=exp(a)exp(b)).
  - O^T[d,i] per head = [V-head | ones] (lhsT) x E^T (rhs): row 64 of the psum
    output is the softmax denominator for free (ones column in V).
  - per head pair: DVE copies the two denominator rows out, one reciprocal,
    one GPSIMD partition_broadcast to [64,2,196]; normalization is fused into
    the O^T PSUM->SBUF eviction (DVE multiply by the broadcast reciprocal).
  - y = O^T chunks (lhsT) x P^T (rhs) + proj_b (DVE add with broadcast bias).
Hardware notes: matmuls at different partition bases must not share a psum
bank (device-fatal); K=64 head-pair matmuls alternate PE row groups 0/64.
"""

import sys

import numpy as np

if "/opt/trn_rl_repo" not in sys.path:
    sys.path.insert(0, "/opt/trn_rl_repo")

import concourse.bass as bass  # noqa: E402
import concourse.mybir as mybir  # noqa: E402
import concourse.tile as tile  # noqa: E402
from concourse import bacc  # noqa: E402
from concourse import bass_utils  # noqa: E402
from concourse.masks import make_identity  # noqa: E402

# Problem shapes (hardcoded; kernel.py must be self-contained).
B, N, C = 128, 196, 768
H, HD = 12, 64
WS = 14
NCORES = 8
BW = B // NCORES  # 16 windows per core
NPAIRS = BW // 2
JC = 98  # j/t chunk size (2 chunks per 196-token window)
F32 = mybir.dt.float32
BF16 = mybir.dt.bfloat16
NP_BF16 = mybir.dt.np(BF16)
SCALE = HD ** -0.5  # 0.125


def _relative_position_index(ws: int) -> np.ndarray:
    coords = np.stack(np.meshgrid(np.arange(ws), np.arange(ws), indexing="ij"))
    flat = coords.reshape(2, -1)
    rel = flat[:, :, None] - flat[:, None, :]
    rel = rel.transpose(1, 2, 0).copy()
    rel[..., 0] += ws - 1
    rel[..., 1] += ws - 1
    rel[..., 0] *= 2 * ws - 1
    return rel.sum(-1)  # [N, N] int


def _build_kernel_body(ctx, tc, aps, reps=1):
    nc = tc.nc
    x_d = aps["x_sh"]
    wT_d = aps["wT"]
    pT_d = aps["pT"]
    qb_d = aps["qb"]
    vb_d = aps["vb"]
    pb_d = aps["pb"]
    eb_d = aps["expBT"]
    y_d = aps["y_sh"]

    const = ctx.enter_context(tc.tile_pool(name="const", bufs=1))

    # ---- resident constants ----
    w_sb = const.tile([128, 6, 3 * C], BF16)  # W^T: [c%128, c//128, c']
    nc.sync.dma_start(out=w_sb, in_=wT_d.rearrange("(a p) m -> p a m", p=128))
    pT_sb = const.tile([128, 6, C], BF16)
    nc.sync.dma_start(out=pT_sb, in_=pT_d.rearrange("(a p) m -> p a m", p=128))
    qb_sb = const.tile([128, 6], F32)
    nc.sync.dma_start(out=qb_sb, in_=qb_d.rearrange("(a p) -> p a", p=128))
    def _bcast(src, parts):
        return bass.AP(tensor=src.tensor, offset=src.offset,
                       ap=[[0, parts]] + list(src.ap))

    vb_bc = const.tile([128, C], F32)  # v_bias broadcast along partitions
    nc.sync.dma_start(out=vb_bc, in_=_bcast(vb_d, 128))
    pb_bc = const.tile([128, C], F32)
    nc.sync.dma_start(out=pb_bc, in_=_bcast(pb_d, 128))
    # 8*bias^T table: [j%98, h, (j//98)*196+i]; injected into the S psum
    # via an identity-copy matmul so exp(0.125*(S+8b)) = exp(S/8 + b)
    b8_sb = const.tile([JC, H, 2 * N], BF16)
    nc.sync.dma_start(out=b8_sb, in_=eb_d.rearrange("p (h m) -> p h m", h=H))
    ident = const.tile([128, 128], BF16)
    make_identity(nc, ident)

    # ---- pools ----
    xin = ctx.enter_context(tc.tile_pool(name="xin", bufs=4))
    xt = ctx.enter_context(tc.tile_pool(name="xt", bufs=2))
    qk = ctx.enter_context(tc.tile_pool(name="qk", bufs=2))
    vpool = ctx.enter_context(tc.tile_pool(name="vpool", bufs=4))
    epool = ctx.enter_context(tc.tile_pool(name="epool", bufs=4))
    opool = ctx.enter_context(tc.tile_pool(name="opool", bufs=4))
    rpool = ctx.enter_context(tc.tile_pool(name="rpool", bufs=4))
    rbc = ctx.enter_context(tc.tile_pool(name="rbc", bufs=4))
    ypool = ctx.enter_context(tc.tile_pool(name="ypool", bufs=2))
    ps_mm = ctx.enter_context(tc.tile_pool(name="ps_mm", bufs=2, space="PSUM"))
    ps_s = ctx.enter_context(tc.tile_pool(name="ps_s", bufs=3, space="PSUM"))
    ps_od = ctx.enter_context(tc.tile_pool(name="ps_od", bufs=3, space="PSUM"))

    # Software pipeline over the global pair stream: while pair P's
    # attention chains run (PE-starved dependency chains), the next pair's
    # transpose/QKV/V matmul groups are woven between them in program order
    # so the in-order PE queue always has independent work.
    state = {}

    def emit_load(P):
        pi = P % NPAIRS
        st = {"wins": (2 * pi, 2 * pi + 1), "xa": [], "xb": []}
        for w in st["wins"]:
            ta = xin.tile([128, C], BF16, tag="xa", name="ta")
            nc.sync.dma_start(out=ta, in_=x_d[w, 0:128, :])
            st["xa"].append(ta)
            tb = xin.tile([128, C], BF16, tag="xb", name="tb")  # rows 0:68
            nc.sync.dma_start(out=tb[0:68, :], in_=x_d[w, 128:196, :])
            st["xb"].append(tb)
        st["xT"] = xt.tile([128, 6, 2 * N], BF16, name="xT")
        st["qk"] = qk.tile([128, 12, 2 * N], BF16, name="qk_sb")
        st["v"] = [
            vpool.tile([128, 2, H, HD + 1], BF16, tag="v", name="vt")
            for _ in range(2)
        ]
        st["oc"] = [
            opool.tile([128, 6, N], BF16, tag="oc", name="oc") for _ in range(2)
        ]
        state[P] = st

    def emit_vms(P, wi):
        nc.gpsimd.memset(state[P]["v"][wi][0:JC, :, :, HD : HD + 1], 1.0)

    def emit_transpose(P, ci):
        st = state[P]
        pt = ps_mm.tile([128, 512], BF16, tag="mm", name="pt")
        for wi in range(2):
            nc.tensor.transpose(
                pt[:, wi * N : wi * N + 128],
                st["xa"][wi][:, ci * 128 : (ci + 1) * 128],
                ident,
            )
            nc.tensor.transpose(
                pt[:, wi * N + 128 : wi * N + N],
                st["xb"][wi][0:68, ci * 128 : (ci + 1) * 128],
                ident[0:68, 0:68],
            )
        nc.scalar.copy(out=st["xT"][:, ci, :], in_=pt[:, 0 : 2 * N])

    def emit_qk(P, cp):
        st = state[P]
        ps = ps_mm.tile([128, 512], F32, tag="mm", name="ps")
        for ck in range(6):
            nc.tensor.matmul(
                ps[:, 0 : 2 * N],
                w_sb[:, ck, cp * 128 : (cp + 1) * 128],
                st["xT"][:, ck, :],
                start=(ck == 0),
                stop=(ck == 5),
            )
        if cp < 6:  # Q: add q_bias (per-partition bias) on ACT
            nc.scalar.activation(
                out=st["qk"][:, cp, :], in_=ps[:, 0 : 2 * N],
                func=mybir.ActivationFunctionType.Identity,
                bias=qb_sb[:, cp : cp + 1],
            )
        else:  # K: plain copy on GPSIMD (InstTensorCopy is library-free)
            nc.gpsimd.tensor_copy(out=st["qk"][:, cp, :], in_=ps[:, 0 : 2 * N])

    def emit_v(P, wi, tck, c0, nn):
        st = state[P]
        ps = ps_mm.tile([128, 512], F32, tag="mm", name="ps")
        for ck in range(6):
            nc.tensor.matmul(
                ps[0:JC, 0:nn],
                st["xT"][:, ck, wi * N + tck * JC : wi * N + (tck + 1) * JC],
                w_sb[:, ck, 2 * C + c0 : 2 * C + c0 + nn],
                start=(ck == 0),
                stop=(ck == 5),
            )
        h0 = c0 // HD
        nh = nn // HD
        nc.vector.tensor_add(
            out=st["v"][wi][0:JC, tck, h0 : h0 + nh, 0:HD],
            in0=ps[0:JC, 0:nn].rearrange("p (h d) -> p h d", d=HD),
            in1=vb_bc[0:JC, c0 : c0 + nn].rearrange("p (h d) -> p h d", d=HD),
        )

    def emit_att_S(P, g, wi):
        st = state[P]
        woff = wi * N
        qk_sb = st["qk"]
        e2 = epool.tile([JC, 2, 2, N], BF16, tag="e", name="e2")  # [j,hh,jc,i]
        # S phase for both heads first, so exp(hh=0) overlaps S(hh=1) and
        # the O matmuls never head-of-line-block the PE queue
        for hh in range(2):
            h = 2 * g + hh
            prow = (h % 2) * 64
            pss = ps_s.tile([128, 512], F32, tag="s", name="pss")
            # seed psum with 8*bias^T (identity-copy matmul, both jc
            # column halves at once), then accumulate S^T on top; exp's
            # 0.125 scale folds both
            nc.tensor.matmul(
                pss[0:JC, 0 : 2 * N],
                ident[0:JC, 0:JC],
                b8_sb[:, h, :],
                start=True,
                stop=False,
            )
            for jc in range(2):
                nc.tensor.matmul(
                    pss[0:JC, jc * N : (jc + 1) * N],
                    qk_sb[prow : prow + 64, 6 + h // 2,
                          woff + jc * JC : woff + (jc + 1) * JC],
                    qk_sb[prow : prow + 64, h // 2, woff : woff + N],
                    start=False,
                    stop=(jc == 1),
                )
            nc.scalar.activation(
                out=e2[:, hh, :, :],
                in_=pss[0:JC, 0 : 2 * N].rearrange("p (a n) -> p a n", a=2),
                func=mybir.ActivationFunctionType.Exp,
                scale=SCALE,
            )
        return e2

    def emit_att_O(P, g, wi, e2):
        st = state[P]
        oc = st["oc"][wi]
        # O^T (+denominator row 64 via the V ones column); both heads
        # share one psum tile (same partition base, disjoint free)
        pso = ps_od.tile([HD + 1, 2, N], F32, tag="od", name="pso")
        for hh in range(2):
            h = 2 * g + hh
            for jc in range(2):
                nc.tensor.matmul(
                    pso[:, hh, :],
                    st["v"][wi][0:JC, jc, h, :],
                    e2[0:JC, hh, jc, :],
                    start=(jc == 0),
                    stop=(jc == 1),
                )
        r2 = rpool.tile([1, 2, N], F32, tag="r", name="r2")
        nc.vector.reciprocal(out=r2, in_=pso[HD : HD + 1, :, :])
        rb = rbc.tile([64, 2, N], F32, tag="rb", name="rb")
        nc.gpsimd.partition_broadcast(rb, r2)
        nc.vector.tensor_mul(oc[0:64, g, :], pso[0:HD, 0, :], rb[:, 0, :])
        nc.vector.tensor_mul(oc[64:128, g, :], pso[0:HD, 1, :], rb[:, 1, :])

    def emit_proj(P, wi, tck):
        st = state[P]
        w = st["wins"][wi]
        oc = st["oc"][wi]
        y_t = ypool.tile([128, C], BF16, tag="y", name="y_t")
        for c0, nn in ((0, 512), (512, 256)):
            ps = ps_mm.tile([128, 512], F32, tag="mm", name="ps")
            for ck in range(6):
                nc.tensor.matmul(
                    ps[0:JC, 0:nn],
                    oc[:, ck, tck * JC : (tck + 1) * JC],
                    pT_sb[:, ck, c0 : c0 + nn],
                    start=(ck == 0),
                    stop=(ck == 5),
                )
            nc.vector.tensor_add(
                out=y_t[0:JC, c0 : c0 + nn],
                in0=ps[0:JC, 0:nn],
                in1=pb_bc[0:JC, c0 : c0 + nn],
            )
        nc.sync.dma_start(
            out=y_d[w, tck * JC : (tck + 1) * JC, :], in_=y_t[0:JC, :]
        )

    def fillers(P):
        fs = []
        for wi in range(2):
            fs.append(lambda wi=wi: emit_vms(P, wi))
        for ci in range(6):
            fs.append(lambda ci=ci: emit_transpose(P, ci))
        for cp in range(12):
            fs.append(lambda cp=cp: emit_qk(P, cp))
        for wi in range(2):
            for tck in range(2):
                for c0, nn in ((0, 512), (512, 256)):
                    fs.append(
                        lambda wi=wi, tck=tck, c0=c0, nn=nn:
                        emit_v(P, wi, tck, c0, nn))
        return fs

    total = reps * NPAIRS
    emit_load(0)
    for f in fillers(0):
        f()
    for P in range(total):
        nf = []
        if P + 1 < total:
            emit_load(P + 1)
            nf = fillers(P + 1)
        chains = [(g, wi) for g in range(6) for wi in range(2)]
        fi = 0

        def drain(upto):
            nonlocal fi
            while fi < min(upto, len(nf)):
                nf[fi]()
                fi += 1

        # two filler slots per chain: between S and O (covers exp latency
        # on the in-order PE queue) and after the tail
        for idx, (g, wi) in enumerate(chains):
            e2 = emit_att_S(P, g, wi)
            drain((2 * idx + 1) * len(nf) // (2 * len(chains)))
            emit_att_O(P, g, wi, e2)
            drain((2 * idx + 2) * len(nf) // (2 * len(chains)))
        drain(len(nf))
        for wi in range(2):
            for tck in range(2):
                emit_proj(P, wi, tck)
        del state[P]


def build_program(reps=1):
    """Build + compile the per-core Bass program. Returns the Bacc instance."""
    nc = bacc.Bacc(
        "TRN2",
        target_bir_lowering=False,
        debug=False,
        enable_asserts=False,
        num_devices=NCORES,
    )
    aps = {
        "x_sh": nc.dram_tensor("x_sh", [BW, N, C], BF16, kind="ExternalInput").ap(),
        "wT": nc.dram_tensor("wT", [C, 3 * C], BF16, kind="ExternalInput").ap(),
        "pT": nc.dram_tensor("pT", [C, C], BF16, kind="ExternalInput").ap(),
        "qb": nc.dram_tensor("qb", [C], F32, kind="ExternalInput").ap(),
        "vb": nc.dram_tensor("vb", [C], F32, kind="ExternalInput").ap(),
        "pb": nc.dram_tensor("pb", [C], F32, kind="ExternalInput").ap(),
        "expBT": nc.dram_tensor(
            "expBT", [JC, H * 2 * N], BF16, kind="ExternalInput").ap(),
        "y_sh": nc.dram_tensor("y_sh", [BW, N, C], BF16, kind="ExternalOutput").ap(),
    }

    from contextlib import ExitStack

    with tile.TileContext(nc) as tc:
        with ExitStack() as ctx:
            _build_kernel_body(ctx, tc, aps, reps=reps)
    nc.compile()
    return nc


_CACHED = {}


def _get_program(reps=1):
    key = f"nc{reps}"
    if key not in _CACHED:
        _CACHED[key] = build_program(reps=reps)
    return _CACHED[key]


def host_prep(qkv_w, q_bias, v_bias, rpb_table, proj_w, proj_b):
    """Host-side constant layout prep (shared across cores)."""
    idx = _relative_position_index(WS)  # [N, N] ints
    bias = rpb_table[idx.reshape(-1)].reshape(N, N, H)  # [i, j, h]
    b8 = 8.0 * bias.astype(np.float32)
    # expBT[r, h, jc*N + i] = 8*bias[i, jc*JC + r, h]
    e = b8.transpose(2, 1, 0).reshape(H, 2, JC, N)  # [h, jc, r, i]
    expBT = np.ascontiguousarray(e.transpose(2, 0, 1, 3)).reshape(JC, H * 2 * N)
    return {
        "wT": np.ascontiguousarray(qkv_w.T).astype(NP_BF16),
        "pT": np.ascontiguousarray(proj_w.T).astype(NP_BF16),
        "qb": np.ascontiguousarray(q_bias, np.float32),
        "vb": np.ascontiguousarray(v_bias, np.float32),
        "pb": np.ascontiguousarray(proj_b, np.float32),
        "expBT": expBT.astype(NP_BF16),
    }


def make_in_maps(x, qkv_w, q_bias, v_bias, rpb_table, proj_w, proj_b):
    shared = host_prep(qkv_w, q_bias, v_bias, rpb_table, proj_w, proj_b)
    in_maps = []
    x_bf = np.asarray(x, np.float32).astype(NP_BF16)
    for ci in range(NCORES):
        m = dict(shared)
        m["x_sh"] = np.ascontiguousarray(x_bf[ci * BW : (ci + 1) * BW])
        in_maps.append(m)
    return in_maps


def kernel(x, qkv_w, q_bias, v_bias, rpb_table, proj_w, proj_b, _trace=False):
    """Full-input entry point: shards over 8 NeuronCores, returns full output."""
    nc = _get_program()
    in_maps = make_in_maps(x, qkv_w, q_bias, v_bias, rpb_table, proj_w, proj_b)
    res = bass_utils.run_bass_kernel_spmd(
        nc, in_maps, core_ids=list(range(NCORES)), trace=_trace)
    out = np.concatenate(
        [res.results[i]["y_sh"] for i in range(NCORES)], axis=0
    ).astype(np.float32)
    if _trace:
        return out, res
    return out



# revision 32
# speedup vs baseline: 2.6513x; 1.0794x over previous
"""Trainium2 Bass kernel for windowed multi-head attention (Swin-style block).

Reference computation (per batch window b of 128, N=196 tokens, C=768, H=12 heads):
    qkv  = x @ qkv_w.T + [q_bias, 0, v_bias]
    q,k,v = split(qkv);  attn = softmax(q*scale @ k.T + rel_pos_bias)
    out  = (attn @ v) @ proj_w.T + proj_b

Sharding: data-parallel over batch across 8 cores (16 windows/core).

Per-core kernel layout strategy (all matmuls consume operands in natural layout,
softmax runs in "transposed" space so no on-the-fly attention transposes):
  - x [196,768] is PE-transposed once to xT [768,196] (lhsT/rhs source).
  - Q^T,K^T [c',t] = W^T chunks (lhsT) x xT (rhs); V [t,c'] = xT (lhsT) x W^T (rhs).
  - S^T[j,i] = K^T-head (lhsT) x Q^T-head (rhs), K=64 contraction; heads pairs
    are row-tiled (partition base 0/64) so two K=64 matmuls share the PE array.
  - E^T = exp(0.125*S^T) * exp(bias)^T  (ACT exp from PSUM, DVE multiply with a
    host-precomputed exp(bias) table -- exp(a+b)=exp(a)exp(b)).
  - O^T[d,i] per head = [V-head | ones] (lhsT) x E^T (rhs): row 64 of the psum
    output is the softmax denominator for free (ones column in V).
  - per head pair: DVE copies the two denominator rows out, one reciprocal,
    one GPSIMD partition_broadcast to [64,2,196]; normalization is fused into
    the O^T PSUM->SBUF eviction (DVE multiply by the broadcast reciprocal).
  - y = O^T chunks (lhsT) x P^T (rhs) + proj_b (DVE add with broadcast bias).
Hardware notes: matmuls at different partition bases must not share a psum
bank (device-fatal); K=64 head-pair matmuls alternate PE row groups 0/64.
"""

import sys

import numpy as np

if "/opt/trn_rl_repo" not in sys.path:
    sys.path.insert(0, "/opt/trn_rl_repo")

import concourse.bass as bass  # noqa: E402
import concourse.mybir as mybir  # noqa: E402
import concourse.tile as tile  # noqa: E402
from concourse import bacc  # noqa: E402
from concourse import bass_utils  # noqa: E402
from concourse.masks import make_identity  # noqa: E402

# Problem shapes (hardcoded; kernel.py must be self-contained).
B, N, C = 128, 196, 768
H, HD = 12, 64
WS = 14
NCORES = 8
BW = B // NCORES  # 16 windows per core
NPAIRS = BW // 2
JC = 98  # j/t chunk size (2 chunks per 196-token window)
F32 = mybir.dt.float32
BF16 = mybir.dt.bfloat16
NP_BF16 = mybir.dt.np(BF16)
SCALE = HD ** -0.5  # 0.125


def _relative_position_index(ws: int) -> np.ndarray:
    coords = np.stack(np.meshgrid(np.arange(ws), np.arange(ws), indexing="ij"))
    flat = coords.reshape(2, -1)
    rel = flat[:, :, None] - flat[:, None, :]
    rel = rel.transpose(1, 2, 0).copy()
    rel[..., 0] += ws - 1
    rel[..., 1] += ws - 1
    rel[..., 0] *= 2 * ws - 1
    return rel.sum(-1)  # [N, N] int


def _build_kernel_body(ctx, tc, aps, reps=1):
    nc = tc.nc
    x_d = aps["x_sh"]
    wT_d = aps["wT"]
    pT_d = aps["pT"]
    qb_d = aps["qb"]
    vb_d = aps["vb"]
    pb_d = aps["pb"]
    eb_d = aps["expBT"]
    y_d = aps["y_sh"]

    const = ctx.enter_context(tc.tile_pool(name="const", bufs=1))

    # ---- resident constants ----
    w_sb = const.tile([128, 6, 3 * C], BF16)  # W^T: [c%128, c//128, c']
    nc.sync.dma_start(out=w_sb, in_=wT_d.rearrange("(a p) m -> p a m", p=128))
    pT_sb = const.tile([128, 6, C], BF16)
    nc.sync.dma_start(out=pT_sb, in_=pT_d.rearrange("(a p) m -> p a m", p=128))
    qb_sb = const.tile([128, 6], F32)
    nc.sync.dma_start(out=qb_sb, in_=qb_d.rearrange("(a p) -> p a", p=128))
    def _bcast(src, parts):
        return bass.AP(tensor=src.tensor, offset=src.offset,
                       ap=[[0, parts]] + list(src.ap))

    vb_bc = const.tile([128, C], F32)  # v_bias broadcast along partitions
    nc.sync.dma_start(out=vb_bc, in_=_bcast(vb_d, 128))
    pb_bc = const.tile([128, C], F32)
    nc.sync.dma_start(out=pb_bc, in_=_bcast(pb_d, 128))
    # 8*bias^T table: [j%98, h, (j//98)*196+i]; injected into the S psum
    # via an identity-copy matmul so exp(0.125*(S+8b)) = exp(S/8 + b)
    b8_sb = const.tile([JC, H, 2 * N], BF16)
    nc.sync.dma_start(out=b8_sb, in_=eb_d.rearrange("p (h m) -> p h m", h=H))
    ident = const.tile([128, 128], BF16)
    make_identity(nc, ident)

    # ---- pools ----
    xin = ctx.enter_context(tc.tile_pool(name="xin", bufs=4))
    xt = ctx.enter_context(tc.tile_pool(name="xt", bufs=2))
    qk = ctx.enter_context(tc.tile_pool(name="qk", bufs=2))
    vpool = ctx.enter_context(tc.tile_pool(name="vpool", bufs=4))
    epool = ctx.enter_context(tc.tile_pool(name="epool", bufs=4))
    opool = ctx.enter_context(tc.tile_pool(name="opool", bufs=4))
    rpool = ctx.enter_context(tc.tile_pool(name="rpool", bufs=4))
    rbc = ctx.enter_context(tc.tile_pool(name="rbc", bufs=4))
    ypool = ctx.enter_context(tc.tile_pool(name="ypool", bufs=2))
    ps_mm = ctx.enter_context(tc.tile_pool(name="ps_mm", bufs=2, space="PSUM"))
    ps_s = ctx.enter_context(tc.tile_pool(name="ps_s", bufs=3, space="PSUM"))
    ps_od = ctx.enter_context(tc.tile_pool(name="ps_od", bufs=3, space="PSUM"))

    # Software pipeline over the global pair stream: while pair P's
    # attention chains run (PE-starved dependency chains), the next pair's
    # transpose/QKV/V matmul groups are woven between them in program order
    # so the in-order PE queue always has independent work.
    state = {}

    def emit_load(P):
        pi = P % NPAIRS
        st = {"wins": (2 * pi, 2 * pi + 1), "xa": [], "xb": []}
        for w in st["wins"]:
            ta = xin.tile([128, C], BF16, tag="xa", name="ta")
            nc.sync.dma_start(out=ta, in_=x_d[w, 0:128, :])
            st["xa"].append(ta)
            tb = xin.tile([128, C], BF16, tag="xb", name="tb")  # rows 0:68
            nc.sync.dma_start(out=tb[0:68, :], in_=x_d[w, 128:196, :])
            st["xb"].append(tb)
        st["xT"] = xt.tile([128, 6, 2 * N], BF16, name="xT")
        st["qk"] = qk.tile([128, 12, 2 * N], BF16, name="qk_sb")
        st["v"] = [
            vpool.tile([128, 2, H, HD + 1], BF16, tag="v", name="vt")
            for _ in range(2)
        ]
        st["oc"] = [
            opool.tile([128, 6, N], BF16, tag="oc", name="oc") for _ in range(2)
        ]
        state[P] = st

    def emit_vms(P, wi):
        nc.gpsimd.memset(state[P]["v"][wi][0:JC, :, :, HD : HD + 1], 1.0)

    def emit_transpose(P, ci):
        st = state[P]
        pt = ps_mm.tile([128, 512], BF16, tag="mm", name="pt")
        for wi in range(2):
            nc.tensor.transpose(
                pt[:, wi * N : wi * N + 128],
                st["xa"][wi][:, ci * 128 : (ci + 1) * 128],
                ident,
            )
            nc.tensor.transpose(
                pt[:, wi * N + 128 : wi * N + N],
                st["xb"][wi][0:68, ci * 128 : (ci + 1) * 128],
                ident[0:68, 0:68],
            )
        nc.scalar.copy(out=st["xT"][:, ci, :], in_=pt[:, 0 : 2 * N])

    def emit_qk(P, cp):
        st = state[P]
        ps = ps_mm.tile([128, 512], F32, tag="mm", name="ps")
        for ck in range(6):
            nc.tensor.matmul(
                ps[:, 0 : 2 * N],
                w_sb[:, ck, cp * 128 : (cp + 1) * 128],
                st["xT"][:, ck, :],
                start=(ck == 0),
                stop=(ck == 5),
            )
        if cp < 6:  # Q: add q_bias (per-partition bias) on ACT
            nc.scalar.activation(
                out=st["qk"][:, cp, :], in_=ps[:, 0 : 2 * N],
                func=mybir.ActivationFunctionType.Identity,
                bias=qb_sb[:, cp : cp + 1],
            )
        else:  # K: plain copy on ACT
            nc.scalar.copy(out=st["qk"][:, cp, :], in_=ps[:, 0 : 2 * N])

    def emit_v(P, wi, tck, c0, nn):
        st = state[P]
        ps = ps_mm.tile([128, 512], F32, tag="mm", name="ps")
        for ck in range(6):
            nc.tensor.matmul(
                ps[0:JC, 0:nn],
                st["xT"][:, ck, wi * N + tck * JC : wi * N + (tck + 1) * JC],
                w_sb[:, ck, 2 * C + c0 : 2 * C + c0 + nn],
                start=(ck == 0),
                stop=(ck == 5),
            )
        h0 = c0 // HD
        nh = nn // HD
        nc.vector.tensor_add(
            out=st["v"][wi][0:JC, tck, h0 : h0 + nh, 0:HD],
            in0=ps[0:JC, 0:nn].rearrange("p (h d) -> p h d", d=HD),
            in1=vb_bc[0:JC, c0 : c0 + nn].rearrange("p (h d) -> p h d", d=HD),
        )

    def emit_att_S(P, g, wi):
        st = state[P]
        woff = wi * N
        qk_sb = st["qk"]
        e2 = epool.tile([JC, 2, 2, N], BF16, tag="e", name="e2")  # [j,hh,jc,i]
        # S phase for both heads first, so exp(hh=0) overlaps S(hh=1) and
        # the O matmuls never head-of-line-block the PE queue
        for hh in range(2):
            h = 2 * g + hh
            prow = (h % 2) * 64
            pss = ps_s.tile([128, 512], F32, tag="s", name="pss")
            # seed psum with 8*bias^T (identity-copy matmul, both jc
            # column halves at once), then accumulate S^T on top; exp's
            # 0.125 scale folds both
            nc.tensor.matmul(
                pss[0:JC, 0 : 2 * N],
                ident[0:JC, 0:JC],
                b8_sb[:, h, :],
                start=True,
                stop=False,
            )
            for jc in range(2):
                nc.tensor.matmul(
                    pss[0:JC, jc * N : (jc + 1) * N],
                    qk_sb[prow : prow + 64, 6 + h // 2,
                          woff + jc * JC : woff + (jc + 1) * JC],
                    qk_sb[prow : prow + 64, h // 2, woff : woff + N],
                    start=False,
                    stop=(jc == 1),
                )
            nc.scalar.activation(
                out=e2[:, hh, :, :],
                in_=pss[0:JC, 0 : 2 * N].rearrange("p (a n) -> p a n", a=2),
                func=mybir.ActivationFunctionType.Exp,
                scale=SCALE,
            )
        return e2

    def emit_att_O(P, g, wi, e2):
        st = state[P]
        oc = st["oc"][wi]
        # O^T (+denominator row 64 via the V ones column); both heads
        # share one psum tile (same partition base, disjoint free)
        pso = ps_od.tile([HD + 1, 2, N], F32, tag="od", name="pso")
        for hh in range(2):
            h = 2 * g + hh
            for jc in range(2):
                nc.tensor.matmul(
                    pso[:, hh, :],
                    st["v"][wi][0:JC, jc, h, :],
                    e2[0:JC, hh, jc, :],
                    start=(jc == 0),
                    stop=(jc == 1),
                )
        r2 = rpool.tile([1, 2, N], F32, tag="r", name="r2")
        nc.vector.reciprocal(out=r2, in_=pso[HD : HD + 1, :, :])
        rb = rbc.tile([64, 2, N], F32, tag="rb", name="rb")
        nc.gpsimd.partition_broadcast(rb, r2)
        nc.vector.tensor_mul(oc[0:64, g, :], pso[0:HD, 0, :], rb[:, 0, :])
        nc.vector.tensor_mul(oc[64:128, g, :], pso[0:HD, 1, :], rb[:, 1, :])

    def emit_proj(P, wi, tck):
        st = state[P]
        w = st["wins"][wi]
        oc = st["oc"][wi]
        y_t = ypool.tile([128, C], BF16, tag="y", name="y_t")
        for c0, nn in ((0, 512), (512, 256)):
            ps = ps_mm.tile([128, 512], F32, tag="mm", name="ps")
            for ck in range(6):
                nc.tensor.matmul(
                    ps[0:JC, 0:nn],
                    oc[:, ck, tck * JC : (tck + 1) * JC],
                    pT_sb[:, ck, c0 : c0 + nn],
                    start=(ck == 0),
                    stop=(ck == 5),
                )
            nc.vector.tensor_add(
                out=y_t[0:JC, c0 : c0 + nn],
                in0=ps[0:JC, 0:nn],
                in1=pb_bc[0:JC, c0 : c0 + nn],
            )
        nc.sync.dma_start(
            out=y_d[w, tck * JC : (tck + 1) * JC, :], in_=y_t[0:JC, :]
        )

    def fillers(P):
        fs = []
        for wi in range(2):
            fs.append(lambda wi=wi: emit_vms(P, wi))
        for ci in range(6):
            fs.append(lambda ci=ci: emit_transpose(P, ci))
        for cp in range(12):
            fs.append(lambda cp=cp: emit_qk(P, cp))
        for wi in range(2):
            for tck in range(2):
                for c0, nn in ((0, 512), (512, 256)):
                    fs.append(
                        lambda wi=wi, tck=tck, c0=c0, nn=nn:
                        emit_v(P, wi, tck, c0, nn))
        return fs

    total = reps * NPAIRS
    emit_load(0)
    for f in fillers(0):
        f()
    for P in range(total):
        nf = []
        if P + 1 < total:
            emit_load(P + 1)
            nf = fillers(P + 1)
        chains = [(g, wi) for g in range(6) for wi in range(2)]
        fi = 0

        def drain(upto):
            nonlocal fi
            while fi < min(upto, len(nf)):
                nf[fi]()
                fi += 1

        # two filler slots per chain: between S and O (covers exp latency
        # on the in-order PE queue) and after the tail
        for idx, (g, wi) in enumerate(chains):
            e2 = emit_att_S(P, g, wi)
            drain((2 * idx + 1) * len(nf) // (2 * len(chains)))
            emit_att_O(P, g, wi, e2)
            drain((2 * idx + 2) * len(nf) // (2 * len(chains)))
        drain(len(nf))
        for wi in range(2):
            for tck in range(2):
                emit_proj(P, wi, tck)
        del state[P]


def build_program(reps=1):
    """Build + compile the per-core Bass program. Returns the Bacc instance."""
    nc = bacc.Bacc(
        "TRN2",
        target_bir_lowering=False,
        debug=False,
        enable_asserts=False,
        num_devices=NCORES,
    )
    aps = {
        "x_sh": nc.dram_tensor("x_sh", [BW, N, C], BF16, kind="ExternalInput").ap(),
        "wT": nc.dram_tensor("wT", [C, 3 * C], BF16, kind="ExternalInput").ap(),
        "pT": nc.dram_tensor("pT", [C, C], BF16, kind="ExternalInput").ap(),
        "qb": nc.dram_tensor("qb", [C], F32, kind="ExternalInput").ap(),
        "vb": nc.dram_tensor("vb", [C], F32, kind="ExternalInput").ap(),
        "pb": nc.dram_tensor("pb", [C], F32, kind="ExternalInput").ap(),
        "expBT": nc.dram_tensor(
            "expBT", [JC, H * 2 * N], BF16, kind="ExternalInput").ap(),
        "y_sh": nc.dram_tensor("y_sh", [BW, N, C], BF16, kind="ExternalOutput").ap(),
    }

    from contextlib import ExitStack

    with tile.TileContext(nc) as tc:
        with ExitStack() as ctx:
            _build_kernel_body(ctx, tc, aps, reps=reps)
    nc.compile()
    return nc


_CACHED = {}


def _get_program(reps=1):
    key = f"nc{reps}"
    if key not in _CACHED:
        _CACHED[key] = build_program(reps=reps)
    return _CACHED[key]


def host_prep(qkv_w, q_bias, v_bias, rpb_table, proj_w, proj_b):
    """Host-side constant layout prep (shared across cores)."""
    idx = _relative_position_index(WS)  # [N, N] ints
    bias = rpb_table[idx.reshape(-1)].reshape(N, N, H)  # [i, j, h]
    b8 = 8.0 * bias.astype(np.float32)
    # expBT[r, h, jc*N + i] = 8*bias[i, jc*JC + r, h]
    e = b8.transpose(2, 1, 0).reshape(H, 2, JC, N)  # [h, jc, r, i]
    expBT = np.ascontiguousarray(e.transpose(2, 0, 1, 3)).reshape(JC, H * 2 * N)
    return {
        "wT": np.ascontiguousarray(qkv_w.T).astype(NP_BF16),
        "pT": np.ascontiguousarray(proj_w.T).astype(NP_BF16),
        "qb": np.ascontiguousarray(q_bias, np.float32),
        "vb": np.ascontiguousarray(v_bias, np.float32),
        "pb": np.ascontiguousarray(proj_b, np.float32),
        "expBT": expBT.astype(NP_BF16),
    }


def make_in_maps(x, qkv_w, q_bias, v_bias, rpb_table, proj_w, proj_b):
    shared = host_prep(qkv_w, q_bias, v_bias, rpb_table, proj_w, proj_b)
    in_maps = []
    x_bf = np.asarray(x, np.float32).astype(NP_BF16)
    for ci in range(NCORES):
        m = dict(shared)
        m["x_sh"] = np.ascontiguousarray(x_bf[ci * BW : (ci + 1) * BW])
        in_maps.append(m)
    return in_maps


def kernel(x, qkv_w, q_bias, v_bias, rpb_table, proj_w, proj_b, _trace=False):
    """Full-input entry point: shards over 8 NeuronCores, returns full output."""
    nc = _get_program()
    in_maps = make_in_maps(x, qkv_w, q_bias, v_bias, rpb_table, proj_w, proj_b)
    res = bass_utils.run_bass_kernel_spmd(
        nc, in_maps, core_ids=list(range(NCORES)), trace=_trace)
    out = np.concatenate(
        [res.results[i]["y_sh"] for i in range(NCORES)], axis=0
    ).astype(np.float32)
    if _trace:
        return out, res
    return out



# revision 33
# speedup vs baseline: 2.7667x; 1.0435x over previous
"""Trainium2 Bass kernel for windowed multi-head attention (Swin-style block).

Reference computation (per batch window b of 128, N=196 tokens, C=768, H=12 heads):
    qkv  = x @ qkv_w.T + [q_bias, 0, v_bias]
    q,k,v = split(qkv);  attn = softmax(q*scale @ k.T + rel_pos_bias)
    out  = (attn @ v) @ proj_w.T + proj_b

Sharding: data-parallel over batch across 8 cores (16 windows/core).

Per-core kernel layout strategy (all matmuls consume operands in natural layout,
softmax runs in "transposed" space so no on-the-fly attention transposes):
  - x [196,768] is PE-transposed once to xT [768,196] (lhsT/rhs source).
  - Q^T,K^T [c',t] = W^T chunks (lhsT) x xT (rhs); V [t,c'] = xT (lhsT) x W^T (rhs).
  - S^T[j,i] = K^T-head (lhsT) x Q^T-head (rhs), K=64 contraction; heads pairs
    are row-tiled (partition base 0/64) so two K=64 matmuls share the PE array.
  - E^T = exp(0.125*S^T) * exp(bias)^T  (ACT exp from PSUM, DVE multiply with a
    host-precomputed exp(bias) table -- exp(a+b)=exp(a)exp(b)).
  - O^T[d,i] per head = [V-head | ones] (lhsT) x E^T (rhs): row 64 of the psum
    output is the softmax denominator for free (ones column in V).
  - per head pair: DVE copies the two denominator rows out, one reciprocal,
    one GPSIMD partition_broadcast to [64,2,196]; normalization is fused into
    the O^T PSUM->SBUF eviction (DVE multiply by the broadcast reciprocal).
  - y = O^T chunks (lhsT) x P^T (rhs) + proj_b (DVE add with broadcast bias).
Hardware notes: matmuls at different partition bases must not share a psum
bank (device-fatal); K=64 head-pair matmuls alternate PE row groups 0/64.
"""

import sys

import numpy as np

if "/opt/trn_rl_repo" not in sys.path:
    sys.path.insert(0, "/opt/trn_rl_repo")

import concourse.bass as bass  # noqa: E402
import concourse.mybir as mybir  # noqa: E402
import concourse.tile as tile  # noqa: E402
from concourse import bacc  # noqa: E402
from concourse import bass_utils  # noqa: E402
from concourse.masks import make_identity  # noqa: E402

# Problem shapes (hardcoded; kernel.py must be self-contained).
B, N, C = 128, 196, 768
H, HD = 12, 64
WS = 14
NCORES = 8
BW = B // NCORES  # 16 windows per core
NPAIRS = BW // 2
JC = 98  # j/t chunk size (2 chunks per 196-token window)
F32 = mybir.dt.float32
BF16 = mybir.dt.bfloat16
NP_BF16 = mybir.dt.np(BF16)
SCALE = HD ** -0.5  # 0.125


def _relative_position_index(ws: int) -> np.ndarray:
    coords = np.stack(np.meshgrid(np.arange(ws), np.arange(ws), indexing="ij"))
    flat = coords.reshape(2, -1)
    rel = flat[:, :, None] - flat[:, None, :]
    rel = rel.transpose(1, 2, 0).copy()
    rel[..., 0] += ws - 1
    rel[..., 1] += ws - 1
    rel[..., 0] *= 2 * ws - 1
    return rel.sum(-1)  # [N, N] int


def _build_kernel_body(ctx, tc, aps, reps=1):
    nc = tc.nc
    x_d = aps["x_sh"]
    wT_d = aps["wT"]
    pT_d = aps["pT"]
    qb_d = aps["qb"]
    vb_d = aps["vb"]
    pb_d = aps["pb"]
    eb_d = aps["expBT"]
    y_d = aps["y_sh"]

    const = ctx.enter_context(tc.tile_pool(name="const", bufs=1))

    # ---- resident constants ----
    w_sb = const.tile([128, 6, 3 * C], BF16)  # W^T: [c%128, c//128, c']
    nc.sync.dma_start(out=w_sb, in_=wT_d.rearrange("(a p) m -> p a m", p=128))
    pT_sb = const.tile([128, 6, C], BF16)
    nc.sync.dma_start(out=pT_sb, in_=pT_d.rearrange("(a p) m -> p a m", p=128))
    qb_sb = const.tile([128, 6], F32)
    nc.sync.dma_start(out=qb_sb, in_=qb_d.rearrange("(a p) -> p a", p=128))
    def _bcast(src, parts):
        return bass.AP(tensor=src.tensor, offset=src.offset,
                       ap=[[0, parts]] + list(src.ap))

    vb_bc = const.tile([128, C], F32)  # v_bias broadcast along partitions
    nc.sync.dma_start(out=vb_bc, in_=_bcast(vb_d, 128))
    pb_bc = const.tile([128, C], F32)
    nc.sync.dma_start(out=pb_bc, in_=_bcast(pb_d, 128))
    # 8*bias^T table: [j%98, h, (j//98)*196+i]; injected into the S psum
    # via an identity-copy matmul so exp(0.125*(S+8b)) = exp(S/8 + b)
    b8_sb = const.tile([JC, H, 2 * N], BF16)
    nc.sync.dma_start(out=b8_sb, in_=eb_d.rearrange("p (h m) -> p h m", h=H))
    ident = const.tile([128, 128], BF16)
    make_identity(nc, ident)

    # ---- pools ----
    xin = ctx.enter_context(tc.tile_pool(name="xin", bufs=4))
    xt = ctx.enter_context(tc.tile_pool(name="xt", bufs=2))
    qk = ctx.enter_context(tc.tile_pool(name="qk", bufs=2))
    vpool = ctx.enter_context(tc.tile_pool(name="vpool", bufs=4))
    epool = ctx.enter_context(tc.tile_pool(name="epool", bufs=4))
    opool = ctx.enter_context(tc.tile_pool(name="opool", bufs=4))
    rpool = ctx.enter_context(tc.tile_pool(name="rpool", bufs=4))
    rbc = ctx.enter_context(tc.tile_pool(name="rbc", bufs=4))
    ypool = ctx.enter_context(tc.tile_pool(name="ypool", bufs=2))
    ps_mm = ctx.enter_context(tc.tile_pool(name="ps_mm", bufs=2, space="PSUM"))
    ps_s = ctx.enter_context(tc.tile_pool(name="ps_s", bufs=3, space="PSUM"))
    ps_od = ctx.enter_context(tc.tile_pool(name="ps_od", bufs=3, space="PSUM"))

    # Software pipeline over the global pair stream: while pair P's
    # attention chains run (PE-starved dependency chains), the next pair's
    # transpose/QKV/V matmul groups are woven between them in program order
    # so the in-order PE queue always has independent work.
    state = {}

    def emit_load(P):
        pi = P % NPAIRS
        st = {"wins": (2 * pi, 2 * pi + 1), "xa": [], "xb": []}
        for w in st["wins"]:
            ta = xin.tile([128, C], BF16, tag="xa", name="ta")
            nc.sync.dma_start(out=ta, in_=x_d[w, 0:128, :])
            st["xa"].append(ta)
            tb = xin.tile([128, C], BF16, tag="xb", name="tb")  # rows 0:68
            nc.sync.dma_start(out=tb[0:68, :], in_=x_d[w, 128:196, :])
            st["xb"].append(tb)
        st["xT"] = xt.tile([128, 6, 2 * N], BF16, name="xT")
        st["qk"] = qk.tile([128, 12, 2 * N], BF16, name="qk_sb")
        st["v"] = [
            vpool.tile([128, 2, H, HD + 1], BF16, tag="v", name="vt")
            for _ in range(2)
        ]
        st["oc"] = [
            opool.tile([128, 6, N], BF16, tag="oc", name="oc") for _ in range(2)
        ]
        state[P] = st

    def emit_vms(P, wi):
        nc.gpsimd.memset(state[P]["v"][wi][0:JC, :, :, HD : HD + 1], 1.0)

    def emit_transpose(P, ci):
        st = state[P]
        pt = ps_mm.tile([128, 512], BF16, tag="mm", name="pt")
        for wi in range(2):
            nc.tensor.transpose(
                pt[:, wi * N : wi * N + 128],
                st["xa"][wi][:, ci * 128 : (ci + 1) * 128],
                ident,
            )
            nc.tensor.transpose(
                pt[:, wi * N + 128 : wi * N + N],
                st["xb"][wi][0:68, ci * 128 : (ci + 1) * 128],
                ident[0:68, 0:68],
            )
        nc.scalar.copy(out=st["xT"][:, ci, :], in_=pt[:, 0 : 2 * N])

    def emit_qk(P, cp):
        st = state[P]
        ps = ps_mm.tile([128, 512], F32, tag="mm", name="ps")
        for ck in range(6):
            nc.tensor.matmul(
                ps[:, 0 : 2 * N],
                w_sb[:, ck, cp * 128 : (cp + 1) * 128],
                st["xT"][:, ck, :],
                start=(ck == 0),
                stop=(ck == 5),
            )
        if cp < 6:  # Q: add q_bias (per-partition bias) on ACT
            nc.scalar.activation(
                out=st["qk"][:, cp, :], in_=ps[:, 0 : 2 * N],
                func=mybir.ActivationFunctionType.Identity,
                bias=qb_sb[:, cp : cp + 1],
            )
        else:  # K: plain copy on ACT
            nc.scalar.copy(out=st["qk"][:, cp, :], in_=ps[:, 0 : 2 * N])

    def emit_v(P, wi, tck, c0, nn):
        st = state[P]
        ps = ps_mm.tile([128, 512], F32, tag="mm", name="ps")
        for ck in range(6):
            nc.tensor.matmul(
                ps[0:JC, 0:nn],
                st["xT"][:, ck, wi * N + tck * JC : wi * N + (tck + 1) * JC],
                w_sb[:, ck, 2 * C + c0 : 2 * C + c0 + nn],
                start=(ck == 0),
                stop=(ck == 5),
            )
        h0 = c0 // HD
        nh = nn // HD
        nc.vector.tensor_add(
            out=st["v"][wi][0:JC, tck, h0 : h0 + nh, 0:HD],
            in0=ps[0:JC, 0:nn].rearrange("p (h d) -> p h d", d=HD),
            in1=vb_bc[0:JC, c0 : c0 + nn].rearrange("p (h d) -> p h d", d=HD),
        )

    def emit_att_S(P, g, wi):
        st = state[P]
        woff = wi * N
        qk_sb = st["qk"]
        e2 = epool.tile([JC, 2, 2, N], BF16, tag="e", name="e2")  # [j,hh,jc,i]
        # S phase for both heads first, so exp(hh=0) overlaps S(hh=1) and
        # the O matmuls never head-of-line-block the PE queue
        for hh in range(2):
            h = 2 * g + hh
            prow = (h % 2) * 64
            pss = ps_s.tile([128, 512], F32, tag="s", name="pss")
            # seed psum with 8*bias^T (identity-copy matmul, both jc
            # column halves at once), then accumulate S^T on top; exp's
            # 0.125 scale folds both
            nc.tensor.matmul(
                pss[0:JC, 0 : 2 * N],
                ident[0:JC, 0:JC],
                b8_sb[:, h, :],
                start=True,
                stop=False,
            )
            for jc in range(2):
                nc.tensor.matmul(
                    pss[0:JC, jc * N : (jc + 1) * N],
                    qk_sb[prow : prow + 64, 6 + h // 2,
                          woff + jc * JC : woff + (jc + 1) * JC],
                    qk_sb[prow : prow + 64, h // 2, woff : woff + N],
                    start=False,
                    stop=(jc == 1),
                )
            nc.scalar.activation(
                out=e2[:, hh, :, :],
                in_=pss[0:JC, 0 : 2 * N].rearrange("p (a n) -> p a n", a=2),
                func=mybir.ActivationFunctionType.Exp,
                scale=SCALE,
            )
        return e2

    def emit_att_O(P, g, wi, e2):
        st = state[P]
        oc = st["oc"][wi]
        # O^T (+denominator row 64 via the V ones column); both heads
        # share one psum tile (same partition base, disjoint free)
        pso = ps_od.tile([HD + 1, 2, N], F32, tag="od", name="pso")
        for hh in range(2):
            h = 2 * g + hh
            for jc in range(2):
                nc.tensor.matmul(
                    pso[:, hh, :],
                    st["v"][wi][0:JC, jc, h, :],
                    e2[0:JC, hh, jc, :],
                    start=(jc == 0),
                    stop=(jc == 1),
                )
        r2 = rpool.tile([1, 2, N], F32, tag="r", name="r2")
        nc.vector.reciprocal(out=r2, in_=pso[HD : HD + 1, :, :])
        rb = rbc.tile([64, 2, N], F32, tag="rb", name="rb")
        nc.gpsimd.partition_broadcast(rb, r2)
        nc.vector.tensor_mul(oc[0:64, g, :], pso[0:HD, 0, :], rb[:, 0, :])
        nc.vector.tensor_mul(oc[64:128, g, :], pso[0:HD, 1, :], rb[:, 1, :])

    def emit_proj(P, wi, tck):
        st = state[P]
        w = st["wins"][wi]
        oc = st["oc"][wi]
        y_t = ypool.tile([128, C], BF16, tag="y", name="y_t")
        for c0, nn in ((0, 512), (512, 256)):
            ps = ps_mm.tile([128, 512], F32, tag="mm", name="ps")
            for ck in range(6):
                nc.tensor.matmul(
                    ps[0:JC, 0:nn],
                    oc[:, ck, tck * JC : (tck + 1) * JC],
                    pT_sb[:, ck, c0 : c0 + nn],
                    start=(ck == 0),
                    stop=(ck == 5),
                )
            nc.vector.tensor_add(
                out=y_t[0:JC, c0 : c0 + nn],
                in0=ps[0:JC, 0:nn],
                in1=pb_bc[0:JC, c0 : c0 + nn],
            )
        nc.sync.dma_start(
            out=y_d[w, tck * JC : (tck + 1) * JC, :], in_=y_t[0:JC, :]
        )

    def fillers(P):
        fs = []
        for wi in range(2):
            fs.append(lambda wi=wi: emit_vms(P, wi))
        for ci in range(6):
            fs.append(lambda ci=ci: emit_transpose(P, ci))
        for cp in range(12):
            fs.append(lambda cp=cp: emit_qk(P, cp))
        for wi in range(2):
            for tck in range(2):
                for c0, nn in ((0, 512), (512, 256)):
                    fs.append(
                        lambda wi=wi, tck=tck, c0=c0, nn=nn:
                        emit_v(P, wi, tck, c0, nn))
        return fs

    total = reps * NPAIRS
    emit_load(0)
    for f in fillers(0):
        f()
    for P in range(total):
        nf = []
        if P + 1 < total:
            emit_load(P + 1)
            nf = fillers(P + 1)
        chains = [(g, wi) for g in range(6) for wi in range(2)]
        fi = 0

        def drain(upto):
            nonlocal fi
            while fi < min(upto, len(nf)):
                nf[fi]()
                fi += 1

        # one filler slot per chain (after the tail)
        for idx, (g, wi) in enumerate(chains):
            e2 = emit_att_S(P, g, wi)
            emit_att_O(P, g, wi, e2)
            drain((idx + 1) * len(nf) // len(chains))
        drain(len(nf))
        for wi in range(2):
            for tck in range(2):
                emit_proj(P, wi, tck)
        del state[P]


def build_program(reps=1):
    """Build + compile the per-core Bass program. Returns the Bacc instance."""
    nc = bacc.Bacc(
        "TRN2",
        target_bir_lowering=False,
        debug=False,
        enable_asserts=False,
        num_devices=NCORES,
    )
    aps = {
        "x_sh": nc.dram_tensor("x_sh", [BW, N, C], BF16, kind="ExternalInput").ap(),
        "wT": nc.dram_tensor("wT", [C, 3 * C], BF16, kind="ExternalInput").ap(),
        "pT": nc.dram_tensor("pT", [C, C], BF16, kind="ExternalInput").ap(),
        "qb": nc.dram_tensor("qb", [C], F32, kind="ExternalInput").ap(),
        "vb": nc.dram_tensor("vb", [C], F32, kind="ExternalInput").ap(),
        "pb": nc.dram_tensor("pb", [C], F32, kind="ExternalInput").ap(),
        "expBT": nc.dram_tensor(
            "expBT", [JC, H * 2 * N], BF16, kind="ExternalInput").ap(),
        "y_sh": nc.dram_tensor("y_sh", [BW, N, C], BF16, kind="ExternalOutput").ap(),
    }

    from contextlib import ExitStack

    with tile.TileContext(nc) as tc:
        with ExitStack() as ctx:
            _build_kernel_body(ctx, tc, aps, reps=reps)
    nc.compile()
    return nc


_CACHED = {}


def _get_program(reps=1):
    key = f"nc{reps}"
    if key not in _CACHED:
        _CACHED[key] = build_program(reps=reps)
    return _CACHED[key]


def host_prep(qkv_w, q_bias, v_bias, rpb_table, proj_w, proj_b):
    """Host-side constant layout prep (shared across cores)."""
    idx = _relative_position_index(WS)  # [N, N] ints
    bias = rpb_table[idx.reshape(-1)].reshape(N, N, H)  # [i, j, h]
    b8 = 8.0 * bias.astype(np.float32)
    # expBT[r, h, jc*N + i] = 8*bias[i, jc*JC + r, h]
    e = b8.transpose(2, 1, 0).reshape(H, 2, JC, N)  # [h, jc, r, i]
    expBT = np.ascontiguousarray(e.transpose(2, 0, 1, 3)).reshape(JC, H * 2 * N)
    return {
        "wT": np.ascontiguousarray(qkv_w.T).astype(NP_BF16),
        "pT": np.ascontiguousarray(proj_w.T).astype(NP_BF16),
        "qb": np.ascontiguousarray(q_bias, np.float32),
        "vb": np.ascontiguousarray(v_bias, np.float32),
        "pb": np.ascontiguousarray(proj_b, np.float32),
        "expBT": expBT.astype(NP_BF16),
    }


def make_in_maps(x, qkv_w, q_bias, v_bias, rpb_table, proj_w, proj_b):
    shared = host_prep(qkv_w, q_bias, v_bias, rpb_table, proj_w, proj_b)
    in_maps = []
    x_bf = np.asarray(x, np.float32).astype(NP_BF16)
    for ci in range(NCORES):
        m = dict(shared)
        m["x_sh"] = np.ascontiguousarray(x_bf[ci * BW : (ci + 1) * BW])
        in_maps.append(m)
    return in_maps


def kernel(x, qkv_w, q_bias, v_bias, rpb_table, proj_w, proj_b, _trace=False):
    """Full-input entry point: shards over 8 NeuronCores, returns full output."""
    nc = _get_program()
    in_maps = make_in_maps(x, qkv_w, q_bias, v_bias, rpb_table, proj_w, proj_b)
    res = bass_utils.run_bass_kernel_spmd(
        nc, in_maps, core_ids=list(range(NCORES)), trace=_trace)
    out = np.concatenate(
        [res.results[i]["y_sh"] for i in range(NCORES)], axis=0
    ).astype(np.float32)
    if _trace:
        return out, res
    return out



# revision 34
# speedup vs baseline: 3.0118x; 1.0886x over previous
"""Trainium2 Bass kernel for windowed multi-head attention (Swin-style block).

Reference computation (per batch window b of 128, N=196 tokens, C=768, H=12 heads):
    qkv  = x @ qkv_w.T + [q_bias, 0, v_bias]
    q,k,v = split(qkv);  attn = softmax(q*scale @ k.T + rel_pos_bias)
    out  = (attn @ v) @ proj_w.T + proj_b

Sharding: data-parallel over batch across 8 cores (16 windows/core).

Per-core kernel layout strategy (all matmuls consume operands in natural layout,
softmax runs in "transposed" space so no on-the-fly attention transposes):
  - x [196,768] is PE-transposed once to xT [768,196] (lhsT/rhs source).
  - Q^T,K^T [c',t] = W^T chunks (lhsT) x xT (rhs); V [t,c'] = xT (lhsT) x W^T (rhs).
  - S^T[j,i] = K^T-head (lhsT) x Q^T-head (rhs), K=64 contraction; heads pairs
    are row-tiled (partition base 0/64) so two K=64 matmuls share the PE array.
  - E^T = exp(0.125*S^T) * exp(bias)^T  (ACT exp from PSUM, DVE multiply with a
    host-precomputed exp(bias) table -- exp(a+b)=exp(a)exp(b)).
  - O^T[d,i] per head = [V-head | ones] (lhsT) x E^T (rhs): row 64 of the psum
    output is the softmax denominator for free (ones column in V).
  - per head pair: DVE copies the two denominator rows out, one reciprocal,
    one GPSIMD partition_broadcast to [64,2,196]; normalization is fused into
    the O^T PSUM->SBUF eviction (DVE multiply by the broadcast reciprocal).
  - y = O^T chunks (lhsT) x P^T (rhs) + proj_b (DVE add with broadcast bias).
Hardware notes: matmuls at different partition bases must not share a psum
bank (device-fatal); K=64 head-pair matmuls alternate PE row groups 0/64.
"""

import sys

import numpy as np

if "/opt/trn_rl_repo" not in sys.path:
    sys.path.insert(0, "/opt/trn_rl_repo")

import concourse.bass as bass  # noqa: E402
import concourse.mybir as mybir  # noqa: E402
import concourse.tile as tile  # noqa: E402
from concourse import bacc  # noqa: E402
from concourse import bass_utils  # noqa: E402
from concourse.masks import make_identity  # noqa: E402

# Problem shapes (hardcoded; kernel.py must be self-contained).
B, N, C = 128, 196, 768
H, HD = 12, 64
WS = 14
NCORES = 8
BW = B // NCORES  # 16 windows per core
NPAIRS = BW // 2
JC = 98  # j/t chunk size (2 chunks per 196-token window)
F32 = mybir.dt.float32
BF16 = mybir.dt.bfloat16
NP_BF16 = mybir.dt.np(BF16)
SCALE = HD ** -0.5  # 0.125


def _relative_position_index(ws: int) -> np.ndarray:
    coords = np.stack(np.meshgrid(np.arange(ws), np.arange(ws), indexing="ij"))
    flat = coords.reshape(2, -1)
    rel = flat[:, :, None] - flat[:, None, :]
    rel = rel.transpose(1, 2, 0).copy()
    rel[..., 0] += ws - 1
    rel[..., 1] += ws - 1
    rel[..., 0] *= 2 * ws - 1
    return rel.sum(-1)  # [N, N] int


def _build_kernel_body(ctx, tc, aps, reps=1):
    nc = tc.nc
    x_d = aps["x_sh"]
    wT_d = aps["wT"]
    pT_d = aps["pT"]
    qb_d = aps["qb"]
    vb_d = aps["vb"]
    pb_d = aps["pb"]
    eb_d = aps["expBT"]
    y_d = aps["y_sh"]

    const = ctx.enter_context(tc.tile_pool(name="const", bufs=1))

    # ---- resident constants ----
    w_sb = const.tile([128, 6, 3 * C], BF16)  # W^T: [c%128, c//128, c']
    nc.sync.dma_start(out=w_sb, in_=wT_d.rearrange("(a p) m -> p a m", p=128))
    pT_sb = const.tile([128, 6, C], BF16)
    nc.sync.dma_start(out=pT_sb, in_=pT_d.rearrange("(a p) m -> p a m", p=128))
    qb_sb = const.tile([128, 6], F32)
    nc.sync.dma_start(out=qb_sb, in_=qb_d.rearrange("(a p) -> p a", p=128))
    def _bcast(src, parts):
        return bass.AP(tensor=src.tensor, offset=src.offset,
                       ap=[[0, parts]] + list(src.ap))

    vb_bc = const.tile([128, C], F32)  # v_bias broadcast along partitions
    nc.sync.dma_start(out=vb_bc, in_=_bcast(vb_d, 128))
    pb_bc = const.tile([128, C], F32)
    nc.sync.dma_start(out=pb_bc, in_=_bcast(pb_d, 128))
    # 8*bias^T table: [j%98, h, (j//98)*196+i]; injected into the S psum
    # via an identity-copy matmul so exp(0.125*(S+8b)) = exp(S/8 + b)
    b8_sb = const.tile([JC, H, 2 * N], BF16)
    nc.sync.dma_start(out=b8_sb, in_=eb_d.rearrange("p (h m) -> p h m", h=H))
    ident = const.tile([128, 128], BF16)
    make_identity(nc, ident)

    # ---- pools ----
    xin = ctx.enter_context(tc.tile_pool(name="xin", bufs=4))
    xt = ctx.enter_context(tc.tile_pool(name="xt", bufs=2))
    qk = ctx.enter_context(tc.tile_pool(name="qk", bufs=2))
    vpool = ctx.enter_context(tc.tile_pool(name="vpool", bufs=4))
    epool = ctx.enter_context(tc.tile_pool(name="epool", bufs=4))
    opool = ctx.enter_context(tc.tile_pool(name="opool", bufs=4))
    rpool = ctx.enter_context(tc.tile_pool(name="rpool", bufs=4))
    rbc = ctx.enter_context(tc.tile_pool(name="rbc", bufs=4))
    ypool = ctx.enter_context(tc.tile_pool(name="ypool", bufs=2))
    ps_mm = ctx.enter_context(tc.tile_pool(name="ps_mm", bufs=3, space="PSUM"))
    ps_s = ctx.enter_context(tc.tile_pool(name="ps_s", bufs=3, space="PSUM"))
    ps_od = ctx.enter_context(tc.tile_pool(name="ps_od", bufs=2, space="PSUM"))

    # Software pipeline over the global pair stream: while pair P's
    # attention chains run (PE-starved dependency chains), the next pair's
    # transpose/QKV/V matmul groups are woven between them in program order
    # so the in-order PE queue always has independent work.
    state = {}

    def emit_load(P):
        pi = P % NPAIRS
        st = {"wins": (2 * pi, 2 * pi + 1), "xa": [], "xb": []}
        for w in st["wins"]:
            ta = xin.tile([128, C], BF16, tag="xa", name="ta")
            nc.sync.dma_start(out=ta, in_=x_d[w, 0:128, :])
            st["xa"].append(ta)
            tb = xin.tile([128, C], BF16, tag="xb", name="tb")  # rows 0:68
            nc.sync.dma_start(out=tb[0:68, :], in_=x_d[w, 128:196, :])
            st["xb"].append(tb)
        st["xT"] = xt.tile([128, 6, 2 * N], BF16, name="xT")
        st["qk"] = qk.tile([128, 12, 2 * N], BF16, name="qk_sb")
        st["v"] = [
            vpool.tile([128, 2, H, HD + 1], BF16, tag="v", name="vt")
            for _ in range(2)
        ]
        st["oc"] = [
            opool.tile([128, 6, N], BF16, tag="oc", name="oc") for _ in range(2)
        ]
        state[P] = st

    def emit_vms(P, wi):
        nc.gpsimd.memset(state[P]["v"][wi][0:JC, :, :, HD : HD + 1], 1.0)

    def emit_transpose(P, ci):
        st = state[P]
        pt = ps_mm.tile([128, 512], BF16, tag="mm", name="pt")
        for wi in range(2):
            nc.tensor.transpose(
                pt[:, wi * N : wi * N + 128],
                st["xa"][wi][:, ci * 128 : (ci + 1) * 128],
                ident,
            )
            nc.tensor.transpose(
                pt[:, wi * N + 128 : wi * N + N],
                st["xb"][wi][0:68, ci * 128 : (ci + 1) * 128],
                ident[0:68, 0:68],
            )
        nc.scalar.copy(out=st["xT"][:, ci, :], in_=pt[:, 0 : 2 * N])

    def emit_qk(P, cp):
        st = state[P]
        ps = ps_mm.tile([128, 512], F32, tag="mm", name="ps")
        for ck in range(6):
            nc.tensor.matmul(
                ps[:, 0 : 2 * N],
                w_sb[:, ck, cp * 128 : (cp + 1) * 128],
                st["xT"][:, ck, :],
                start=(ck == 0),
                stop=(ck == 5),
            )
        if cp < 6:  # Q: add q_bias (per-partition bias) on ACT
            nc.scalar.activation(
                out=st["qk"][:, cp, :], in_=ps[:, 0 : 2 * N],
                func=mybir.ActivationFunctionType.Identity,
                bias=qb_sb[:, cp : cp + 1],
            )
        else:  # K: plain copy on ACT
            nc.scalar.copy(out=st["qk"][:, cp, :], in_=ps[:, 0 : 2 * N])

    def emit_v(P, wi, tck, c0, nn):
        st = state[P]
        ps = ps_mm.tile([128, 512], F32, tag="mm", name="ps")
        for ck in range(6):
            nc.tensor.matmul(
                ps[0:JC, 0:nn],
                st["xT"][:, ck, wi * N + tck * JC : wi * N + (tck + 1) * JC],
                w_sb[:, ck, 2 * C + c0 : 2 * C + c0 + nn],
                start=(ck == 0),
                stop=(ck == 5),
            )
        h0 = c0 // HD
        nh = nn // HD
        nc.vector.tensor_add(
            out=st["v"][wi][0:JC, tck, h0 : h0 + nh, 0:HD],
            in0=ps[0:JC, 0:nn].rearrange("p (h d) -> p h d", d=HD),
            in1=vb_bc[0:JC, c0 : c0 + nn].rearrange("p (h d) -> p h d", d=HD),
        )

    def emit_att_S(P, g, wi):
        st = state[P]
        woff = wi * N
        qk_sb = st["qk"]
        e2 = epool.tile([JC, 2, 2, N], BF16, tag="e", name="e2")  # [j,hh,jc,i]
        # S phase for both heads first, so exp(hh=0) overlaps S(hh=1) and
        # the O matmuls never head-of-line-block the PE queue
        for hh in range(2):
            h = 2 * g + hh
            prow = (h % 2) * 64
            pss = ps_s.tile([128, 512], F32, tag="s", name="pss")
            # seed psum with 8*bias^T (identity-copy matmul, both jc
            # column halves at once), then accumulate S^T on top; exp's
            # 0.125 scale folds both
            nc.tensor.matmul(
                pss[0:JC, 0 : 2 * N],
                ident[0:JC, 0:JC],
                b8_sb[:, h, :],
                start=True,
                stop=False,
            )
            for jc in range(2):
                nc.tensor.matmul(
                    pss[0:JC, jc * N : (jc + 1) * N],
                    qk_sb[prow : prow + 64, 6 + h // 2,
                          woff + jc * JC : woff + (jc + 1) * JC],
                    qk_sb[prow : prow + 64, h // 2, woff : woff + N],
                    start=False,
                    stop=(jc == 1),
                )
            nc.scalar.activation(
                out=e2[:, hh, :, :],
                in_=pss[0:JC, 0 : 2 * N].rearrange("p (a n) -> p a n", a=2),
                func=mybir.ActivationFunctionType.Exp,
                scale=SCALE,
            )
        return e2

    def emit_att_O(P, g, wi, e2):
        st = state[P]
        oc = st["oc"][wi]
        # O^T (+denominator row 64 via the V ones column); both heads
        # share one psum tile (same partition base, disjoint free)
        pso = ps_od.tile([HD + 1, 2, N], F32, tag="od", name="pso")
        for hh in range(2):
            h = 2 * g + hh
            for jc in range(2):
                nc.tensor.matmul(
                    pso[:, hh, :],
                    st["v"][wi][0:JC, jc, h, :],
                    e2[0:JC, hh, jc, :],
                    start=(jc == 0),
                    stop=(jc == 1),
                )
        r2 = rpool.tile([1, 2, N], F32, tag="r", name="r2")
        nc.vector.reciprocal(out=r2, in_=pso[HD : HD + 1, :, :])
        rb = rbc.tile([64, 2, N], F32, tag="rb", name="rb")
        nc.gpsimd.partition_broadcast(rb, r2)
        nc.vector.tensor_mul(oc[0:64, g, :], pso[0:HD, 0, :], rb[:, 0, :])
        nc.vector.tensor_mul(oc[64:128, g, :], pso[0:HD, 1, :], rb[:, 1, :])

    def emit_proj(P, wi, tck):
        st = state[P]
        w = st["wins"][wi]
        oc = st["oc"][wi]
        y_t = ypool.tile([128, C], BF16, tag="y", name="y_t")
        for c0, nn in ((0, 512), (512, 256)):
            ps = ps_mm.tile([128, 512], F32, tag="mm", name="ps")
            for ck in range(6):
                nc.tensor.matmul(
                    ps[0:JC, 0:nn],
                    oc[:, ck, tck * JC : (tck + 1) * JC],
                    pT_sb[:, ck, c0 : c0 + nn],
                    start=(ck == 0),
                    stop=(ck == 5),
                )
            nc.vector.tensor_add(
                out=y_t[0:JC, c0 : c0 + nn],
                in0=ps[0:JC, 0:nn],
                in1=pb_bc[0:JC, c0 : c0 + nn],
            )
        nc.sync.dma_start(
            out=y_d[w, tck * JC : (tck + 1) * JC, :], in_=y_t[0:JC, :]
        )

    def fillers(P):
        fs = []
        for wi in range(2):
            fs.append(lambda wi=wi: emit_vms(P, wi))
        for ci in range(6):
            fs.append(lambda ci=ci: emit_transpose(P, ci))
        for cp in range(12):
            fs.append(lambda cp=cp: emit_qk(P, cp))
        for wi in range(2):
            for tck in range(2):
                for c0, nn in ((0, 512), (512, 256)):
                    fs.append(
                        lambda wi=wi, tck=tck, c0=c0, nn=nn:
                        emit_v(P, wi, tck, c0, nn))
        return fs

    total = reps * NPAIRS
    emit_load(0)
    for f in fillers(0):
        f()
    for P in range(total):
        nf = []
        if P + 1 < total:
            emit_load(P + 1)
            nf = fillers(P + 1)
        chains = [(g, wi) for g in range(6) for wi in range(2)]
        fi = 0

        def drain(upto):
            nonlocal fi
            while fi < min(upto, len(nf)):
                nf[fi]()
                fi += 1

        # one filler slot per chain (after the tail)
        for idx, (g, wi) in enumerate(chains):
            e2 = emit_att_S(P, g, wi)
            emit_att_O(P, g, wi, e2)
            drain((idx + 1) * len(nf) // len(chains))
        drain(len(nf))
        for wi in range(2):
            for tck in range(2):
                emit_proj(P, wi, tck)
        del state[P]


def build_program(reps=1):
    """Build + compile the per-core Bass program. Returns the Bacc instance."""
    nc = bacc.Bacc(
        "TRN2",
        target_bir_lowering=False,
        debug=False,
        enable_asserts=False,
        num_devices=NCORES,
    )
    aps = {
        "x_sh": nc.dram_tensor("x_sh", [BW, N, C], BF16, kind="ExternalInput").ap(),
        "wT": nc.dram_tensor("wT", [C, 3 * C], BF16, kind="ExternalInput").ap(),
        "pT": nc.dram_tensor("pT", [C, C], BF16, kind="ExternalInput").ap(),
        "qb": nc.dram_tensor("qb", [C], F32, kind="ExternalInput").ap(),
        "vb": nc.dram_tensor("vb", [C], F32, kind="ExternalInput").ap(),
        "pb": nc.dram_tensor("pb", [C], F32, kind="ExternalInput").ap(),
        "expBT": nc.dram_tensor(
            "expBT", [JC, H * 2 * N], BF16, kind="ExternalInput").ap(),
        "y_sh": nc.dram_tensor("y_sh", [BW, N, C], BF16, kind="ExternalOutput").ap(),
    }

    from contextlib import ExitStack

    with tile.TileContext(nc) as tc:
        with ExitStack() as ctx:
            _build_kernel_body(ctx, tc, aps, reps=reps)
    nc.compile()
    return nc


_CACHED = {}


def _get_program(reps=1):
    key = f"nc{reps}"
    if key not in _CACHED:
        _CACHED[key] = build_program(reps=reps)
    return _CACHED[key]


def host_prep(qkv_w, q_bias, v_bias, rpb_table, proj_w, proj_b):
    """Host-side constant layout prep (shared across cores)."""
    idx = _relative_position_index(WS)  # [N, N] ints
    bias = rpb_table[idx.reshape(-1)].reshape(N, N, H)  # [i, j, h]
    b8 = 8.0 * bias.astype(np.float32)
    # expBT[r, h, jc*N + i] = 8*bias[i, jc*JC + r, h]
    e = b8.transpose(2, 1, 0).reshape(H, 2, JC, N)  # [h, jc, r, i]
    expBT = np.ascontiguousarray(e.transpose(2, 0, 1, 3)).reshape(JC, H * 2 * N)
    return {
        "wT": np.ascontiguousarray(qkv_w.T).astype(NP_BF16),
        "pT": np.ascontiguousarray(proj_w.T).astype(NP_BF16),
        "qb": np.ascontiguousarray(q_bias, np.float32),
        "vb": np.ascontiguousarray(v_bias, np.float32),
        "pb": np.ascontiguousarray(proj_b, np.float32),
        "expBT": expBT.astype(NP_BF16),
    }


def make_in_maps(x, qkv_w, q_bias, v_bias, rpb_table, proj_w, proj_b):
    shared = host_prep(qkv_w, q_bias, v_bias, rpb_table, proj_w, proj_b)
    in_maps = []
    x_bf = np.asarray(x, np.float32).astype(NP_BF16)
    for ci in range(NCORES):
        m = dict(shared)
        m["x_sh"] = np.ascontiguousarray(x_bf[ci * BW : (ci + 1) * BW])
        in_maps.append(m)
    return in_maps


def kernel(x, qkv_w, q_bias, v_bias, rpb_table, proj_w, proj_b, _trace=False):
    """Full-input entry point: shards over 8 NeuronCores, returns full output."""
    nc = _get_program()
    in_maps = make_in_maps(x, qkv_w, q_bias, v_bias, rpb_table, proj_w, proj_b)
    res = bass_utils.run_bass_kernel_spmd(
        nc, in_maps, core_ids=list(range(NCORES)), trace=_trace)
    out = np.concatenate(
        [res.results[i]["y_sh"] for i in range(NCORES)], axis=0
    ).astype(np.float32)
    if _trace:
        return out, res
    return out

